# revision 35
# baseline (speedup 1.0000x reference)
# Trainium2 Bass kernel for GPT-J-style cosine attention (no softmax).
#
# Reference computation (B=2, S=1024, E=2048, H=16, HD=128, ROT=64):
#   q/k/v = hs @ W.T ; partial rotary on first 64 dims of each head;
#   v /= max(count^sigmoid(norm_const), 1); q,k L2-normalized; q,k,v
#   masked by attention_mask==0 rows; attn = tril(q @ k.T) (zeros, no
#   softmax); out = (attn @ v) @ w_o.T.
#
# Sharding: core c = b*4 + g  (b in 0..1 batch, g in 0..3 head-group of
# 4 heads). Each core computes its batch's S x 512 slice of q/k/v, runs
# attention for its 4 heads, and produces a partial [S, E] out-proj
# contribution; the host sums the 4 partials per batch.
#
# Optimizations over the fp32r baseline (232us -> 186us):
# - bf16 data path end to end (PSUM accumulation stays fp32): same PE
#   rate as fp32r at >=256-wide moving operands, but half the HBM/DMA
#   bytes and half the SBUF footprint; bf16 partial outputs summed on
#   the host in fp32. rel_err ~6e-3 vs the 2e-2 gate.
# - host pre-packs hs/weights into exact SBUF layout so every load is a
#   2D-contiguous DMA (cheap descriptor generation); hs+wq stream in
#   need-order interleaved 2-k-slice chunks on the sync queue, wk/wv/wo
#   follow on the same queue; small consts ride the gpsimd queue.
# - Q-projection emitted as two k-outer 4-block sweeps that track the
#   DMA stream; K-projection m-outer; all Q/K transposes deferred until
#   after the K matmuls (PE fills the postproc latency window instead
#   of idling; qn ring holds all 16 blocks).
# - V-projection, attention (512-query causal halves), and out-proj
#   interleaved per query half so PSUM evictions on Vector/Scalar hide
#   under PE matmul work; out-proj quarters stream to HBM as evicted,
#   alternating sync/gpsimd queues.
# - two 4-bank PSUM rings (psA/psB) assigned per phase so ring-reuse
#   WAR dependencies never gate the PE; PE warmup on an on-chip const
#   (no DMA dependency) opens the clock gate before real work arrives.
import numpy as np
import ml_dtypes

BF16NP = ml_dtypes.bfloat16

B, S, E, H, HD, ROT, MAXP = 2, 1024, 2048, 16, 128, 64, 2048
HL = 4            # heads per core
GD = HL * HD      # 512 output dims per core
NB = S // 128     # 8 s-blocks
NK = E // 128     # 16 contraction tiles
NC4 = S // 256    # 4 query quarters for attention
EPS = 1e-12


def _sinusoidal(num_pos, dim):
    inv_freq = 1.0 / (10000.0 ** (np.arange(0, dim, 2, dtype=np.float32) / dim))
    sinusoid = np.einsum("i,j->ij", np.arange(num_pos, dtype=np.float32), inv_freq)
    return np.concatenate([np.sin(sinusoid), np.cos(sinusoid)], axis=-1)


_BUILT = None


def _build():
    global _BUILT
    if _BUILT is not None:
        return _BUILT
    import concourse.bacc as bacc
    import concourse.mybir as mybir
    from concourse.tile import TileContext

    F32 = mybir.dt.float32
    BF16 = mybir.dt.bfloat16
    MUL = mybir.AluOpType.mult
    ADD = mybir.AluOpType.add
    SQUARE = mybir.ActivationFunctionType.Square

    nc = bacc.Bacc(None, target_bir_lowering=False)

    hsT = nc.dram_tensor("hsT", [128, NK * S], BF16, kind="ExternalInput")
    wqT = nc.dram_tensor("wqT", [128, NK * GD], BF16, kind="ExternalInput")
    wkT = nc.dram_tensor("wkT", [128, NK * GD], BF16, kind="ExternalInput")
    wvT = nc.dram_tensor("wvT", [128, NK * GD], BF16, kind="ExternalInput")
    woT = nc.dram_tensor("woT", [128, 4 * E], BF16, kind="ExternalInput")
    cos4d = nc.dram_tensor("cos4", [128, NB, HL, ROT], BF16, kind="ExternalInput")
    sin4d = nc.dram_tensor("sin4", [128, NB, HL, ROT], BF16, kind="ExternalInput")
    masksd = nc.dram_tensor("masks", [128, 4, 512], BF16, kind="ExternalInput")
    vscaled = nc.dram_tensor("vscale", [128, NB, HL], F32, kind="ExternalInput")
    qmaskd = nc.dram_tensor("qmask", [128, NB], F32, kind="ExternalInput")
    identd = nc.dram_tensor("ident", [128, 128], BF16, kind="ExternalInput")
    outd = nc.dram_tensor("out", [S, E], BF16, kind="ExternalOutput")

    with TileContext(nc) as tc:
        from contextlib import ExitStack
        ctx = ExitStack()
        with ctx:
            const = ctx.enter_context(tc.tile_pool(name="const", bufs=1))
            qkT_pool = ctx.enter_context(tc.tile_pool(name="qkT", bufs=1))
            vn_pool = ctx.enter_context(tc.tile_pool(name="vn", bufs=1))
            scr = ctx.enter_context(tc.tile_pool(name="scr", bufs=4))
            rot_pool = ctx.enter_context(tc.tile_pool(name="rot", bufs=5))
            # two 4-bank PSUM rings shared by all phases
            psA = ctx.enter_context(tc.tile_pool(name="psA", bufs=4, space="PSUM"))
            psB = ctx.enter_context(tc.tile_pool(name="psB", bufs=4, space="PSUM"))

            cos4 = const.tile([128, NB, HL, ROT], BF16)
            sin4 = const.tile([128, NB, HL, ROT], BF16)
            masks = const.tile([128, 4, 512], BF16)
            vscale = const.tile([128, NB, HL], F32)
            qmask = const.tile([128, NB], F32)
            ident = const.tile([128, 128], BF16)
            # consts on the gpsimd DMA queue; weights go on the scalar
            # queue and hs on the sync queue so the three streams enqueue
            # in parallel.
            nc.gpsimd.dma_start(out=ident[:], in_=identd[:])
            nc.gpsimd.dma_start(out=qmask[:], in_=qmaskd[:])
            nc.gpsimd.dma_start(out=cos4[:, 0:4], in_=cos4d[:, 0:4])
            nc.gpsimd.dma_start(out=sin4[:, 0:4], in_=sin4d[:, 0:4])

            # HAM warmup: keep PE busy on dummy matmuls over an on-chip
            # constant (no DMA dependency) so the clock gate opens to
            # 2.4 GHz before the DMA-paced Q-projection starts.
            ones = nc.const_aps.scalar_like(1.0, qmask[:, 0:1])
            warm_ps = psB.tile([128, 128], F32, tag="ps")
            for _ in range(48):
                nc.tensor.matmul(warm_ps[0:1, 0:1], ones, ones,
                                 start=True, stop=True)

            # persistent transposed q/k: per local head, [hd=128, S]
            qT = [qkT_pool.tile([128, S], BF16, name=f"qT{h}") for h in range(HL)]
            kT = [qkT_pool.tile([128, S], BF16, name=f"kT{h}") for h in range(HL)]
            # v in natural layout per s-block: [128, 512]
            vn = [vn_pool.tile([128, GD], BF16, name=f"vn{m}") for m in range(NB)]
            # attention output (transposed) per head: [hd=128, S]
            aT = [qkT_pool.tile([128, S], BF16, name=f"aT{h}") for h in range(HL)]

            with tc.tile_pool(name="hs", bufs=1) as hs_pool, \
                 tc.tile_pool(name="w", bufs=1) as w_pool, \
                 tc.tile_pool(name="wo", bufs=4) as wo_pool, \
                 tc.tile_pool(name="atn", bufs=12) as atn_pool, \
                 tc.tile_pool(name="ost", bufs=2) as ost_pool:
                import concourse.bass as bass
                hs = hs_pool.tile([128, NK * S], BF16)

                # hs + wq interleaved in need-order on the sync queue, as
                # 4-k-slice chunks; dram is pre-packed in SBUF layout so
                # every DMA is 2D-contiguous (cheap descriptor generation)
                wqt = w_pool.tile([128, NK, GD], BF16, name="wqt", tag="w")
                for j in range(8):
                    nc.sync.dma_start(out=hs[:, j * 2 * S:(j + 1) * 2 * S],
                                      in_=hsT[:, j * 2 * S:(j + 1) * 2 * S])
                    nc.sync.dma_start(out=wqt[:, 2 * j:2 * (j + 1)],
                                      in_=wqT[:, j * 2 * GD:(j + 1) * 2 * GD])
                # then K/V/O weights, in need-order on the same queue
                wkt = w_pool.tile([128, NK, GD], BF16, name="wkt", tag="w")
                nc.sync.dma_start(out=wkt[:], in_=wkT[:])
                wvt = w_pool.tile([128, NK, GD], BF16, name="wvt", tag="w")
                nc.sync.dma_start(out=wvt[:], in_=wvT[:])
                wot = wo_pool.tile([128, 4, 4, 512], BF16, name="wot")
                nc.sync.dma_start(out=wot[:], in_=woT[:])
                wo_tiles = [[wot[:, n, kk] for kk in range(4)] for n in range(4)]
                wq = [wqt[:, k] for k in range(NK)]
                wk = [wkt[:, k] for k in range(NK)]
                wv = [wvt[:, k] for k in range(NK)]

                def proj_mms(wtiles, m, pool):
                    ps = pool.tile([128, GD], F32, name="ps_proj", tag="ps")
                    for k in range(NK):
                        nc.tensor.matmul(
                            ps[:], hs[:, k * S + m * 128: k * S + (m + 1) * 128],
                            wtiles[k], start=(k == 0), stop=(k == NK - 1))
                    return ps

                def qk_postproc(ps, m):
                    # sum-of-squares per head (rotary is norm-preserving, so
                    # norms come pre-rotary, straight from PSUM)
                    ss = scr.tile([128, HL], F32, tag="ss")
                    sqs = scr.tile([128, 128], F32, tag="sqs", bufs=1)
                    for h in range(HL):
                        nc.scalar.activation(out=sqs[:],
                                             in_=ps[:, h * 128:(h + 1) * 128],
                                             func=SQUARE, accum_out=ss[:, h:h + 1])
                    nrm = scr.tile([128, HL], F32, tag="nrm")
                    nc.scalar.sqrt(nrm[:], ss[:])
                    rr = scr.tile([128, HL], F32, tag="rr")
                    nc.vector.reciprocal(rr[:], nrm[:])
                    nc.vector.tensor_scalar_mul(rr[:], rr[:], qmask[:, m:m + 1])
                    # evict PSUM -> SBUF (bf16) with the per-row scale
                    # folded in, on Vector (Scalar is the postproc pacer)
                    qn = rot_pool.tile([128, HL, 128], BF16, tag="qn", bufs=16)
                    for h in range(HL):
                        nc.vector.tensor_scalar_mul(
                            qn[:, h], ps[:, h * 128:(h + 1) * 128], rr[:, h:h + 1])
                    # GPT-J interleaved rotary on first ROT dims of each head
                    qrot = rot_pool.tile([128, HL, ROT], BF16, tag="qrot", bufs=2)
                    tmp2 = rot_pool.tile([128, HL, ROT], BF16, tag="tmp2", bufs=2)
                    nc.gpsimd.tensor_tensor(out=qrot[:, :, 0:ROT:2], in0=qn[:, :, 1:ROT:2],
                                            in1=sin4[:, m, :, 0:ROT:2], op=MUL)
                    nc.vector.tensor_tensor(out=qrot[:, :, 1:ROT:2], in0=qn[:, :, 0:ROT:2],
                                            in1=sin4[:, m, :, 1:ROT:2], op=MUL)
                    nc.gpsimd.tensor_tensor(out=tmp2[:], in0=qn[:, :, 0:ROT],
                                            in1=cos4[:, m], op=MUL)
                    nc.gpsimd.tensor_add(out=qn[:, :, 0:ROT], in0=qrot[:], in1=tmp2[:])
                    return (qn, None)

                def transpose_block(qndg, m, dstT, pool):
                    qn, dg = qndg
                    for h in range(HL):
                        pt = pool.tile([128, 128], BF16, name="pt", tag="ps")
                        nc.tensor.transpose(pt[:], qn[:, h], ident[:])
                        nc.vector.tensor_copy(dstT[h][:, m * 128:(m + 1) * 128], pt[:])

                # ---- Q projection: two k-outer sweeps of 4 m-blocks each,
                # so the PE consumes hs/wq tiles as the DMA stream lands.
                qns = {}
                ps1 = [psA.tile([128, GD], F32, name=f"ps1_{m}", tag="ps")
                       for m in range(4)]
                for k in range(NK):
                    for m in range(4):
                        nc.tensor.matmul(
                            ps1[m][:], hs[:, k * S + m * 128: k * S + (m + 1) * 128],
                            wq[k], start=(k == 0), stop=(k == NK - 1))
                for m in range(4):
                    qns[m] = qk_postproc(ps1[m], m)
                # late consts: not needed before ~30us; deferring their
                # transfers keeps early DMA bandwidth on the hs/wq stream
                nc.gpsimd.dma_start(out=cos4[:, 4:], in_=cos4d[:, 4:])
                nc.gpsimd.dma_start(out=sin4[:, 4:], in_=sin4d[:, 4:])
                nc.gpsimd.dma_start(out=vscale[:], in_=vscaled[:])
                nc.gpsimd.dma_start(out=masks[:], in_=masksd[:])
                for m in range(4, NB):
                    qns[m] = qk_postproc(proj_mms(wq, m, psB), m)
                # ---- K projection (transposes for both Q and K deferred
                # until after all K matmuls: the PE fills the postproc
                # latency window with matmuls instead of idling)
                kns = {}
                for m in range(NB):
                    kns[m] = qk_postproc(proj_mms(wk, m, psA), m)
                for m in range(NB):
                    transpose_block(qns.pop(m), m, qT, psB)
                for m in range(NB - 3):
                    transpose_block(kns.pop(m), m, kT, psB)
                for m in range(NB - 3, NB):
                    transpose_block(kns.pop(m), m, kT, psB)

                # ---- interleaved V-projection / attention / out-projection,
                # one 512-query half at a time
                for c in range(2):
                    # V-proj for s-blocks 4c..4c+3
                    for m in range(4 * c, 4 * c + 4):
                        ps = proj_mms(wv, m, psB)
                        for h in range(HL):
                            if h < 2:
                                nc.vector.tensor_scalar_mul(
                                    vn[m][:, h * 128:(h + 1) * 128],
                                    ps[:, h * 128:(h + 1) * 128],
                                    vscale[:, m, h:h + 1])
                            else:
                                nc.scalar.mul(
                                    vn[m][:, h * 128:(h + 1) * 128],
                                    ps[:, h * 128:(h + 1) * 128],
                                    vscale[:, m, h:h + 1])
                    # attention for query half c, all local heads
                    nj = 4 * (c + 1)
                    for h in range(HL):
                        at_tiles = []
                        for j in range(nj):
                            pa = psA.tile([128, 512], F32, name="pa", tag="ps")
                            nc.tensor.matmul(pa[:], kT[h][:, j * 128:(j + 1) * 128],
                                             qT[h][:, c * 512:(c + 1) * 512],
                                             start=True, stop=True)
                            at = atn_pool.tile([128, 512], BF16, tag="at", name="at")
                            d = j - 4 * c
                            if d >= 0:   # diagonal band: apply causal mask
                                nc.vector.tensor_tensor(out=at[:], in0=pa[:],
                                                        in1=masks[:, d], op=MUL)
                            else:         # below diagonal: plain evict
                                nc.scalar.copy(at[:], pa[:])
                            at_tiles.append(at)
                        po = psB.tile([128, 512], F32, name="po", tag="ps")
                        for j in range(nj):
                            nc.tensor.matmul(po[:], vn[j][:, h * 128:(h + 1) * 128],
                                             at_tiles[j][:],
                                             start=(j == 0), stop=(j == nj - 1))
                        nc.scalar.copy(aT[h][:, c * 512:(c + 1) * 512], po[:])
                    # out-projection for s-blocks 4c..4c+3
                    for m in range(4 * c, 4 * c + 4):
                        ot = ost_pool.tile([128, E], BF16, tag="ot", name="ot")
                        for n in range(4):
                            pool = psA if n % 2 == 0 else psB
                            ps = pool.tile([128, 512], F32, name="ps_o", tag="ps")
                            for k in range(HL):
                                nc.tensor.matmul(ps[:], aT[k][:, m * 128:(m + 1) * 128],
                                                 wo_tiles[n][k],
                                                 start=(k == 0), stop=(k == HL - 1))
                            if n % 2 == 0:
                                nc.vector.tensor_copy(ot[:, n * 512:(n + 1) * 512], ps[:])
                            else:
                                nc.scalar.copy(ot[:, n * 512:(n + 1) * 512], ps[:])
                            # stream each quarter out as soon as it evicts
                            q = nc.sync if n % 2 == 0 else nc.gpsimd
                            q.dma_start(
                                out=outd[m * 128:(m + 1) * 128,
                                         n * 512:(n + 1) * 512],
                                in_=ot[:, n * 512:(n + 1) * 512])

    nc.compile()
    _BUILT = nc
    return nc


def _prep_inputs(hidden_states, w_q, w_k, w_v, w_o, norm_const,
                 attention_mask, position_ids):
    """Host-side shard + table prep. Returns list of 8 in_maps."""
    hidden_states = np.asarray(hidden_states, dtype=np.float32)
    w_q = np.asarray(w_q, dtype=np.float32)
    w_k = np.asarray(w_k, dtype=np.float32)
    w_v = np.asarray(w_v, dtype=np.float32)
    w_o = np.asarray(w_o, dtype=np.float32)
    norm_const = np.asarray(norm_const, dtype=np.float32).reshape(H)
    attention_mask = np.asarray(attention_mask, dtype=np.float32).reshape(B, S)
    position_ids = np.asarray(position_ids).reshape(B, S).astype(np.int64)

    embed = _sinusoidal(MAXP, ROT)                       # [MAXP, 64]
    sig = 1.0 / (1.0 + np.exp(-norm_const.astype(np.float64)))   # [H]
    mask0 = (attention_mask == 0).astype(np.float32)     # [B, S]
    counts = np.cumsum(mask0, axis=1).astype(np.float32)  # [B, S]
    denom = np.maximum(counts[:, None, :] ** sig[None, :, None], 1.0).astype(np.float32)
    vs_full = mask0[:, None, :] / denom                  # [B, H, S]

    # causal masks for the 4 diagonal-band block offsets (512-query cols)
    p = np.arange(128)[:, None]
    f = np.arange(512)[None, :]
    masks = np.stack([(d * 128 + p <= f) for d in range(4)]).astype(BF16NP)
    masks = np.ascontiguousarray(masks.transpose(1, 0, 2))  # [128, 4, 512]
    ident = np.eye(128, dtype=BF16NP)

    in_maps = []
    for b in range(B):
        sincos = embed[position_ids[b]]                  # [S, 64]
        sin, cos = sincos[:, :ROT // 2], sincos[:, ROT // 2:]
        cosR = np.repeat(cos, 2, axis=1)                 # [S, 64]
        sinS = np.empty((S, ROT), dtype=np.float32)
        sinS[:, 0::2] = -sin
        sinS[:, 1::2] = sin
        # [S,64] -> [128 part, NB, 64] -> broadcast over HL heads
        def to4(t):
            t = t.reshape(NB, 128, ROT).transpose(1, 0, 2)
            return np.ascontiguousarray(
                np.broadcast_to(t[:, :, None, :], (128, NB, HL, ROT))).astype(BF16NP)
        cos4 = to4(cosR)
        sin4 = to4(sinS)
        qm = np.ascontiguousarray(mask0[b].reshape(NB, 128).T)  # [128, NB]
        # pack [E, S] -> [128, NK*S] (partition-major SBUF layout)
        hsT_b = np.ascontiguousarray(
            hidden_states[b].T.reshape(NK, 128, S).transpose(1, 0, 2)
            .reshape(128, NK * S)).astype(BF16NP)

        def packw(wT):  # [E, GD] -> [128, NK*GD]
            return np.ascontiguousarray(
                wT.reshape(NK, 128, GD).transpose(1, 0, 2)
                .reshape(128, NK * GD)).astype(BF16NP)

        for g in range(4):
            sl = slice(g * GD, (g + 1) * GD)
            vs = vs_full[b, 4 * g:4 * g + HL, :]                # [HL, S]
            vs = np.ascontiguousarray(
                vs.reshape(HL, NB, 128).transpose(2, 1, 0))     # [128, NB, HL]
            # wo: [GD, E] -> [128, n, kk, 512] -> [128, 4*E]
            woT_g = w_o[:, sl].T                                # [GD, E]
            wo_p = np.ascontiguousarray(
                woT_g.reshape(4, 128, 4, 512).transpose(1, 2, 0, 3)
                .reshape(128, 4 * E)).astype(BF16NP)
            in_maps.append({
                "hsT": hsT_b,
                "wqT": packw(w_q[sl, :].T),
                "wkT": packw(w_k[sl, :].T),
                "wvT": packw(w_v[sl, :].T),
                "woT": wo_p,
                "cos4": cos4, "sin4": sin4, "masks": masks,
                "vscale": vs, "qmask": qm, "ident": ident,
            })
    # core order: c = b*4 + g
    return in_maps


def run(inputs, trace=False, trace_cores=None):
    from concourse.bass_utils import run_bass_kernel_spmd
    nc = _build()
    in_maps = _prep_inputs(**inputs)
    res = run_bass_kernel_spmd(nc, in_maps, core_ids=list(range(8)),
                               trace=trace, trace_cores=trace_cores)
    partials = [np.asarray(res.results[c]["out"], dtype=np.float32)
                for c in range(8)]
    out = np.empty((B, S, E), dtype=np.float32)
    for b in range(B):
        out[b] = partials[4 * b] + partials[4 * b + 1] \
            + partials[4 * b + 2] + partials[4 * b + 3]
    return out, res


def kernel(**inputs):
    out, _ = run(inputs, trace=False)
    return out


# revision 37
# speedup vs baseline: 1.0045x; 1.0045x over previous
# Trainium2 Bass kernel for GPT-J-style cosine attention (no softmax).
#
# Reference computation (B=2, S=1024, E=2048, H=16, HD=128, ROT=64):
#   q/k/v = hs @ W.T ; partial rotary on first 64 dims of each head;
#   v /= max(count^sigmoid(norm_const), 1); q,k L2-normalized; q,k,v
#   masked by attention_mask==0 rows; attn = tril(q @ k.T) (zeros, no
#   softmax); out = (attn @ v) @ w_o.T.
#
# Sharding: core c = b*4 + g  (b in 0..1 batch, g in 0..3 head-group of
# 4 heads). Each core computes its batch's S x 512 slice of q/k/v, runs
# attention for its 4 heads, and produces a partial [S, E] out-proj
# contribution; the host sums the 4 partials per batch.
#
# Optimizations over the fp32r baseline (232us -> 186us):
# - bf16 data path end to end (PSUM accumulation stays fp32): same PE
#   rate as fp32r at >=256-wide moving operands, but half the HBM/DMA
#   bytes and half the SBUF footprint; bf16 partial outputs summed on
#   the host in fp32. rel_err ~6e-3 vs the 2e-2 gate.
# - host pre-packs hs/weights into exact SBUF layout so every load is a
#   2D-contiguous DMA (cheap descriptor generation); hs+wq stream in
#   need-order interleaved 2-k-slice chunks on the sync queue, wk/wv/wo
#   follow on the same queue; small consts ride the gpsimd queue.
# - Q-projection emitted as two k-outer 4-block sweeps that track the
#   DMA stream; K-projection m-outer; all Q/K transposes deferred until
#   after the K matmuls (PE fills the postproc latency window instead
#   of idling; qn ring holds all 16 blocks).
# - V-projection, attention (512-query causal halves), and out-proj
#   interleaved per query half so PSUM evictions on Vector/Scalar hide
#   under PE matmul work; out-proj quarters stream to HBM as evicted,
#   alternating sync/gpsimd queues.
# - two 4-bank PSUM rings (psA/psB) assigned per phase so ring-reuse
#   WAR dependencies never gate the PE; PE warmup on an on-chip const
#   (no DMA dependency) opens the clock gate before real work arrives.
import numpy as np
import ml_dtypes

BF16NP = ml_dtypes.bfloat16

B, S, E, H, HD, ROT, MAXP = 2, 1024, 2048, 16, 128, 64, 2048
HL = 4            # heads per core
GD = HL * HD      # 512 output dims per core
NB = S // 128     # 8 s-blocks
NK = E // 128     # 16 contraction tiles
NC4 = S // 256    # 4 query quarters for attention
EPS = 1e-12


def _sinusoidal(num_pos, dim):
    inv_freq = 1.0 / (10000.0 ** (np.arange(0, dim, 2, dtype=np.float32) / dim))
    sinusoid = np.einsum("i,j->ij", np.arange(num_pos, dtype=np.float32), inv_freq)
    return np.concatenate([np.sin(sinusoid), np.cos(sinusoid)], axis=-1)


_BUILT = None


def _build():
    global _BUILT
    if _BUILT is not None:
        return _BUILT
    import concourse.bacc as bacc
    import concourse.mybir as mybir
    from concourse.tile import TileContext

    F32 = mybir.dt.float32
    BF16 = mybir.dt.bfloat16
    MUL = mybir.AluOpType.mult
    ADD = mybir.AluOpType.add
    SQUARE = mybir.ActivationFunctionType.Square

    nc = bacc.Bacc(None, target_bir_lowering=False)

    hsT = nc.dram_tensor("hsT", [128, NK * S], BF16, kind="ExternalInput")
    wqT = nc.dram_tensor("wqT", [128, NK * GD], BF16, kind="ExternalInput")
    wkT = nc.dram_tensor("wkT", [128, NK * GD], BF16, kind="ExternalInput")
    wvT = nc.dram_tensor("wvT", [128, NK * GD], BF16, kind="ExternalInput")
    woT = nc.dram_tensor("woT", [128, 4 * E], BF16, kind="ExternalInput")
    cos4d = nc.dram_tensor("cos4", [128, NB, HL, ROT], BF16, kind="ExternalInput")
    sin4d = nc.dram_tensor("sin4", [128, NB, HL, ROT], BF16, kind="ExternalInput")
    masksd = nc.dram_tensor("masks", [128, 4, 512], BF16, kind="ExternalInput")
    vscaled = nc.dram_tensor("vscale", [128, NB, HL], F32, kind="ExternalInput")
    qmaskd = nc.dram_tensor("qmask", [128, NB], F32, kind="ExternalInput")
    identd = nc.dram_tensor("ident", [128, 128], BF16, kind="ExternalInput")
    outd = nc.dram_tensor("out", [S, E], BF16, kind="ExternalOutput")

    with TileContext(nc) as tc:
        from contextlib import ExitStack
        ctx = ExitStack()
        with ctx:
            const = ctx.enter_context(tc.tile_pool(name="const", bufs=1))
            qkT_pool = ctx.enter_context(tc.tile_pool(name="qkT", bufs=1))
            vn_pool = ctx.enter_context(tc.tile_pool(name="vn", bufs=1))
            scr = ctx.enter_context(tc.tile_pool(name="scr", bufs=4))
            rot_pool = ctx.enter_context(tc.tile_pool(name="rot", bufs=5))
            # two 4-bank PSUM rings shared by all phases
            psA = ctx.enter_context(tc.tile_pool(name="psA", bufs=4, space="PSUM"))
            psB = ctx.enter_context(tc.tile_pool(name="psB", bufs=4, space="PSUM"))

            cos4 = const.tile([128, NB, HL, ROT], BF16)
            sin4 = const.tile([128, NB, HL, ROT], BF16)
            masks = const.tile([128, 4, 512], BF16)
            vscale = const.tile([128, NB, HL], F32)
            qmask = const.tile([128, NB], F32)
            ident = const.tile([128, 128], BF16)
            # consts on the gpsimd DMA queue; weights go on the scalar
            # queue and hs on the sync queue so the three streams enqueue
            # in parallel.
            nc.gpsimd.dma_start(out=ident[:], in_=identd[:])
            nc.gpsimd.dma_start(out=qmask[:], in_=qmaskd[:])
            nc.gpsimd.dma_start(out=vscale[:], in_=vscaled[:])
            nc.gpsimd.dma_start(out=cos4[:], in_=cos4d[:])
            nc.gpsimd.dma_start(out=sin4[:], in_=sin4d[:])
            nc.gpsimd.dma_start(out=masks[:], in_=masksd[:])

            # HAM warmup: keep PE busy on dummy matmuls over an on-chip
            # constant (no DMA dependency) so the clock gate opens to
            # 2.4 GHz before the DMA-paced Q-projection starts.
            ones = nc.const_aps.scalar_like(1.0, qmask[:, 0:1])
            warm_ps = psB.tile([128, 128], F32, tag="ps")
            for _ in range(48):
                nc.tensor.matmul(warm_ps[0:1, 0:1], ones, ones,
                                 start=True, stop=True)

            # persistent transposed q/k: per local head, [hd=128, S]
            qT = [qkT_pool.tile([128, S], BF16, name=f"qT{h}") for h in range(HL)]
            kT = [qkT_pool.tile([128, S], BF16, name=f"kT{h}") for h in range(HL)]
            # v in natural layout per s-block: [128, 512]
            vn = [vn_pool.tile([128, GD], BF16, name=f"vn{m}") for m in range(NB)]
            # attention output (transposed) per head: [hd=128, S]
            aT = [qkT_pool.tile([128, S], BF16, name=f"aT{h}") for h in range(HL)]

            with tc.tile_pool(name="hs", bufs=1) as hs_pool, \
                 tc.tile_pool(name="w", bufs=1) as w_pool, \
                 tc.tile_pool(name="wo", bufs=4) as wo_pool, \
                 tc.tile_pool(name="atn", bufs=12) as atn_pool, \
                 tc.tile_pool(name="ost", bufs=2) as ost_pool:
                import concourse.bass as bass
                hs = hs_pool.tile([128, NK * S], BF16)

                # hs + wq interleaved in need-order on the sync queue, as
                # 4-k-slice chunks; dram is pre-packed in SBUF layout so
                # every DMA is 2D-contiguous (cheap descriptor generation)
                wqt = w_pool.tile([128, NK, GD], BF16, name="wqt", tag="w")
                for j in range(8):
                    nc.sync.dma_start(out=hs[:, j * 2 * S:(j + 1) * 2 * S],
                                      in_=hsT[:, j * 2 * S:(j + 1) * 2 * S])
                    nc.sync.dma_start(out=wqt[:, 2 * j:2 * (j + 1)],
                                      in_=wqT[:, j * 2 * GD:(j + 1) * 2 * GD])
                # then K/V/O weights, in need-order on the same queue
                wkt = w_pool.tile([128, NK, GD], BF16, name="wkt", tag="w")
                nc.sync.dma_start(out=wkt[:], in_=wkT[:])
                wvt = w_pool.tile([128, NK, GD], BF16, name="wvt", tag="w")
                nc.sync.dma_start(out=wvt[:], in_=wvT[:])
                wot = wo_pool.tile([128, 4, 4, 512], BF16, name="wot")
                nc.sync.dma_start(out=wot[:], in_=woT[:])
                wo_tiles = [[wot[:, n, kk] for kk in range(4)] for n in range(4)]
                wq = [wqt[:, k] for k in range(NK)]
                wk = [wkt[:, k] for k in range(NK)]
                wv = [wvt[:, k] for k in range(NK)]

                def proj_mms(wtiles, m, pool):
                    ps = pool.tile([128, GD], F32, name="ps_proj", tag="ps")
                    for k in range(NK):
                        nc.tensor.matmul(
                            ps[:], hs[:, k * S + m * 128: k * S + (m + 1) * 128],
                            wtiles[k], start=(k == 0), stop=(k == NK - 1))
                    return ps

                def qk_postproc(ps, m):
                    # sum-of-squares per head (rotary is norm-preserving, so
                    # norms come pre-rotary, straight from PSUM)
                    ss = scr.tile([128, HL], F32, tag="ss")
                    sqs = scr.tile([128, 128], F32, tag="sqs", bufs=1)
                    for h in range(HL):
                        nc.scalar.activation(out=sqs[:],
                                             in_=ps[:, h * 128:(h + 1) * 128],
                                             func=SQUARE, accum_out=ss[:, h:h + 1])
                    nrm = scr.tile([128, HL], F32, tag="nrm")
                    nc.scalar.sqrt(nrm[:], ss[:])
                    rr = scr.tile([128, HL], F32, tag="rr")
                    nc.vector.reciprocal(rr[:], nrm[:])
                    nc.vector.tensor_scalar_mul(rr[:], rr[:], qmask[:, m:m + 1])
                    # evict PSUM -> SBUF (bf16) with the per-row scale
                    # folded in, on Vector (Scalar is the postproc pacer)
                    qn = rot_pool.tile([128, HL, 128], BF16, tag="qn", bufs=16)
                    for h in range(HL):
                        nc.vector.tensor_scalar_mul(
                            qn[:, h], ps[:, h * 128:(h + 1) * 128], rr[:, h:h + 1])
                    # GPT-J interleaved rotary on first ROT dims of each head
                    qrot = rot_pool.tile([128, HL, ROT], BF16, tag="qrot", bufs=2)
                    tmp2 = rot_pool.tile([128, HL, ROT], BF16, tag="tmp2", bufs=2)
                    nc.gpsimd.tensor_tensor(out=qrot[:, :, 0:ROT:2], in0=qn[:, :, 1:ROT:2],
                                            in1=sin4[:, m, :, 0:ROT:2], op=MUL)
                    nc.vector.tensor_tensor(out=qrot[:, :, 1:ROT:2], in0=qn[:, :, 0:ROT:2],
                                            in1=sin4[:, m, :, 1:ROT:2], op=MUL)
                    nc.gpsimd.tensor_tensor(out=tmp2[:], in0=qn[:, :, 0:ROT],
                                            in1=cos4[:, m], op=MUL)
                    nc.gpsimd.tensor_add(out=qn[:, :, 0:ROT], in0=qrot[:], in1=tmp2[:])
                    return (qn, None)

                def transpose_block(qndg, m, dstT, pool):
                    qn, dg = qndg
                    for h in range(HL):
                        pt = pool.tile([128, 128], BF16, name="pt", tag="ps")
                        nc.tensor.transpose(pt[:], qn[:, h], ident[:])
                        nc.vector.tensor_copy(dstT[h][:, m * 128:(m + 1) * 128], pt[:])

                # ---- Q projection: one 8-block k-outer sweep across all
                # eight PSUM banks, so the PE consumes the hs/wq DMA chunks
                # at stream rate (32 matmuls per 2-k chunk) with no
                # separate second sweep.
                qns = {}
                ps1 = [(psA if m < 4 else psB).tile(
                           [128, GD], F32, name=f"ps1_{m}", tag="ps")
                       for m in range(NB)]
                for k in range(NK):
                    for m in range(NB):
                        nc.tensor.matmul(
                            ps1[m][:], hs[:, k * S + m * 128: k * S + (m + 1) * 128],
                            wq[k], start=(k == 0), stop=(k == NK - 1))
                for m in range(NB):
                    qns[m] = qk_postproc(ps1[m], m)
                # ---- K projection (transposes for both Q and K deferred
                # until after all K matmuls: the PE fills the postproc
                # latency window with matmuls instead of idling)
                kns = {}
                for m in range(NB):
                    kns[m] = qk_postproc(proj_mms(wk, m, psA), m)
                for m in range(NB):
                    transpose_block(qns.pop(m), m, qT, psB)
                for m in range(NB - 3):
                    transpose_block(kns.pop(m), m, kT, psB)
                for m in range(NB - 3, NB):
                    transpose_block(kns.pop(m), m, kT, psB)

                # ---- interleaved V-projection / attention / out-projection,
                # one 512-query half at a time
                for c in range(2):
                    # V-proj for s-blocks 4c..4c+3
                    for m in range(4 * c, 4 * c + 4):
                        ps = proj_mms(wv, m, psB)
                        for h in range(HL):
                            if h < 2:
                                nc.vector.tensor_scalar_mul(
                                    vn[m][:, h * 128:(h + 1) * 128],
                                    ps[:, h * 128:(h + 1) * 128],
                                    vscale[:, m, h:h + 1])
                            else:
                                nc.scalar.mul(
                                    vn[m][:, h * 128:(h + 1) * 128],
                                    ps[:, h * 128:(h + 1) * 128],
                                    vscale[:, m, h:h + 1])
                    # attention for query half c, all local heads
                    nj = 4 * (c + 1)
                    for h in range(HL):
                        at_tiles = []
                        for j in range(nj):
                            pa = psA.tile([128, 512], F32, name="pa", tag="ps")
                            nc.tensor.matmul(pa[:], kT[h][:, j * 128:(j + 1) * 128],
                                             qT[h][:, c * 512:(c + 1) * 512],
                                             start=True, stop=True)
                            at = atn_pool.tile([128, 512], BF16, tag="at", name="at")
                            d = j - 4 * c
                            if d >= 0:   # diagonal band: apply causal mask
                                nc.vector.tensor_tensor(out=at[:], in0=pa[:],
                                                        in1=masks[:, d], op=MUL)
                            else:         # below diagonal: plain evict
                                nc.scalar.copy(at[:], pa[:])
                            at_tiles.append(at)
                        po = psB.tile([128, 512], F32, name="po", tag="ps")
                        for j in range(nj):
                            nc.tensor.matmul(po[:], vn[j][:, h * 128:(h + 1) * 128],
                                             at_tiles[j][:],
                                             start=(j == 0), stop=(j == nj - 1))
                        nc.scalar.copy(aT[h][:, c * 512:(c + 1) * 512], po[:])
                    # out-projection for s-blocks 4c..4c+3
                    for m in range(4 * c, 4 * c + 4):
                        ot = ost_pool.tile([128, E], BF16, tag="ot", name="ot")
                        for n in range(4):
                            pool = psA if n % 2 == 0 else psB
                            ps = pool.tile([128, 512], F32, name="ps_o", tag="ps")
                            for k in range(HL):
                                nc.tensor.matmul(ps[:], aT[k][:, m * 128:(m + 1) * 128],
                                                 wo_tiles[n][k],
                                                 start=(k == 0), stop=(k == HL - 1))
                            if n % 2 == 0:
                                nc.vector.tensor_copy(ot[:, n * 512:(n + 1) * 512], ps[:])
                            else:
                                nc.scalar.copy(ot[:, n * 512:(n + 1) * 512], ps[:])
                            # stream each quarter out as soon as it evicts
                            q = nc.sync if n % 2 == 0 else nc.gpsimd
                            q.dma_start(
                                out=outd[m * 128:(m + 1) * 128,
                                         n * 512:(n + 1) * 512],
                                in_=ot[:, n * 512:(n + 1) * 512])

    nc.compile()
    _BUILT = nc
    return nc


def _prep_inputs(hidden_states, w_q, w_k, w_v, w_o, norm_const,
                 attention_mask, position_ids):
    """Host-side shard + table prep. Returns list of 8 in_maps."""
    hidden_states = np.asarray(hidden_states, dtype=np.float32)
    w_q = np.asarray(w_q, dtype=np.float32)
    w_k = np.asarray(w_k, dtype=np.float32)
    w_v = np.asarray(w_v, dtype=np.float32)
    w_o = np.asarray(w_o, dtype=np.float32)
    norm_const = np.asarray(norm_const, dtype=np.float32).reshape(H)
    attention_mask = np.asarray(attention_mask, dtype=np.float32).reshape(B, S)
    position_ids = np.asarray(position_ids).reshape(B, S).astype(np.int64)

    embed = _sinusoidal(MAXP, ROT)                       # [MAXP, 64]
    sig = 1.0 / (1.0 + np.exp(-norm_const.astype(np.float64)))   # [H]
    mask0 = (attention_mask == 0).astype(np.float32)     # [B, S]
    counts = np.cumsum(mask0, axis=1).astype(np.float32)  # [B, S]
    denom = np.maximum(counts[:, None, :] ** sig[None, :, None], 1.0).astype(np.float32)
    vs_full = mask0[:, None, :] / denom                  # [B, H, S]

    # causal masks for the 4 diagonal-band block offsets (512-query cols)
    p = np.arange(128)[:, None]
    f = np.arange(512)[None, :]
    masks = np.stack([(d * 128 + p <= f) for d in range(4)]).astype(BF16NP)
    masks = np.ascontiguousarray(masks.transpose(1, 0, 2))  # [128, 4, 512]
    ident = np.eye(128, dtype=BF16NP)

    in_maps = []
    for b in range(B):
        sincos = embed[position_ids[b]]                  # [S, 64]
        sin, cos = sincos[:, :ROT // 2], sincos[:, ROT // 2:]
        cosR = np.repeat(cos, 2, axis=1)                 # [S, 64]
        sinS = np.empty((S, ROT), dtype=np.float32)
        sinS[:, 0::2] = -sin
        sinS[:, 1::2] = sin
        # [S,64] -> [128 part, NB, 64] -> broadcast over HL heads
        def to4(t):
            t = t.reshape(NB, 128, ROT).transpose(1, 0, 2)
            return np.ascontiguousarray(
                np.broadcast_to(t[:, :, None, :], (128, NB, HL, ROT))).astype(BF16NP)
        cos4 = to4(cosR)
        sin4 = to4(sinS)
        qm = np.ascontiguousarray(mask0[b].reshape(NB, 128).T)  # [128, NB]
        # pack [E, S] -> [128, NK*S] (partition-major SBUF layout)
        hsT_b = np.ascontiguousarray(
            hidden_states[b].T.reshape(NK, 128, S).transpose(1, 0, 2)
            .reshape(128, NK * S)).astype(BF16NP)

        def packw(wT):  # [E, GD] -> [128, NK*GD]
            return np.ascontiguousarray(
                wT.reshape(NK, 128, GD).transpose(1, 0, 2)
                .reshape(128, NK * GD)).astype(BF16NP)

        for g in range(4):
            sl = slice(g * GD, (g + 1) * GD)
            vs = vs_full[b, 4 * g:4 * g + HL, :]                # [HL, S]
            vs = np.ascontiguousarray(
                vs.reshape(HL, NB, 128).transpose(2, 1, 0))     # [128, NB, HL]
            # wo: [GD, E] -> [128, n, kk, 512] -> [128, 4*E]
            woT_g = w_o[:, sl].T                                # [GD, E]
            wo_p = np.ascontiguousarray(
                woT_g.reshape(4, 128, 4, 512).transpose(1, 2, 0, 3)
                .reshape(128, 4 * E)).astype(BF16NP)
            in_maps.append({
                "hsT": hsT_b,
                "wqT": packw(w_q[sl, :].T),
                "wkT": packw(w_k[sl, :].T),
                "wvT": packw(w_v[sl, :].T),
                "woT": wo_p,
                "cos4": cos4, "sin4": sin4, "masks": masks,
                "vscale": vs, "qmask": qm, "ident": ident,
            })
    # core order: c = b*4 + g
    return in_maps


def run(inputs, trace=False, trace_cores=None):
    from concourse.bass_utils import run_bass_kernel_spmd
    nc = _build()
    in_maps = _prep_inputs(**inputs)
    res = run_bass_kernel_spmd(nc, in_maps, core_ids=list(range(8)),
                               trace=trace, trace_cores=trace_cores)
    partials = [np.asarray(res.results[c]["out"], dtype=np.float32)
                for c in range(8)]
    out = np.empty((B, S, E), dtype=np.float32)
    for b in range(B):
        out[b] = partials[4 * b] + partials[4 * b + 1] \
            + partials[4 * b + 2] + partials[4 * b + 3]
    return out, res


def kernel(**inputs):
    out, _ = run(inputs, trace=False)
    return out


# revision 38
# speedup vs baseline: 1.0064x; 1.0018x over previous
# Trainium2 Bass kernel for GPT-J-style cosine attention (no softmax).
#
# Reference computation (B=2, S=1024, E=2048, H=16, HD=128, ROT=64):
#   q/k/v = hs @ W.T ; partial rotary on first 64 dims of each head;
#   v /= max(count^sigmoid(norm_const), 1); q,k L2-normalized; q,k,v
#   masked by attention_mask==0 rows; attn = tril(q @ k.T) (zeros, no
#   softmax); out = (attn @ v) @ w_o.T.
#
# Sharding: core c = b*4 + g  (b in 0..1 batch, g in 0..3 head-group of
# 4 heads). Each core computes its batch's S x 512 slice of q/k/v, runs
# attention for its 4 heads, and produces a partial [S, E] out-proj
# contribution; the host sums the 4 partials per batch.
#
# Optimizations over the fp32r baseline (232us -> 186us):
# - bf16 data path end to end (PSUM accumulation stays fp32): same PE
#   rate as fp32r at >=256-wide moving operands, but half the HBM/DMA
#   bytes and half the SBUF footprint; bf16 partial outputs summed on
#   the host in fp32. rel_err ~6e-3 vs the 2e-2 gate.
# - host pre-packs hs/weights into exact SBUF layout so every load is a
#   2D-contiguous DMA (cheap descriptor generation); hs+wq stream in
#   need-order interleaved 2-k-slice chunks on the sync queue, wk/wv/wo
#   follow on the same queue; small consts ride the gpsimd queue.
# - Q-projection emitted as two k-outer 4-block sweeps that track the
#   DMA stream; K-projection m-outer; all Q/K transposes deferred until
#   after the K matmuls (PE fills the postproc latency window instead
#   of idling; qn ring holds all 16 blocks).
# - V-projection, attention (512-query causal halves), and out-proj
#   interleaved per query half so PSUM evictions on Vector/Scalar hide
#   under PE matmul work; out-proj quarters stream to HBM as evicted,
#   alternating sync/gpsimd queues.
# - two 4-bank PSUM rings (psA/psB) assigned per phase so ring-reuse
#   WAR dependencies never gate the PE; PE warmup on an on-chip const
#   (no DMA dependency) opens the clock gate before real work arrives.
import numpy as np
import ml_dtypes

BF16NP = ml_dtypes.bfloat16

B, S, E, H, HD, ROT, MAXP = 2, 1024, 2048, 16, 128, 64, 2048
HL = 4            # heads per core
GD = HL * HD      # 512 output dims per core
NB = S // 128     # 8 s-blocks
NK = E // 128     # 16 contraction tiles
NC4 = S // 256    # 4 query quarters for attention
EPS = 1e-12


def _sinusoidal(num_pos, dim):
    inv_freq = 1.0 / (10000.0 ** (np.arange(0, dim, 2, dtype=np.float32) / dim))
    sinusoid = np.einsum("i,j->ij", np.arange(num_pos, dtype=np.float32), inv_freq)
    return np.concatenate([np.sin(sinusoid), np.cos(sinusoid)], axis=-1)


_BUILT = None


def _build():
    global _BUILT
    if _BUILT is not None:
        return _BUILT
    import concourse.bacc as bacc
    import concourse.mybir as mybir
    from concourse.tile import TileContext

    F32 = mybir.dt.float32
    BF16 = mybir.dt.bfloat16
    MUL = mybir.AluOpType.mult
    ADD = mybir.AluOpType.add
    SQUARE = mybir.ActivationFunctionType.Square

    nc = bacc.Bacc(None, target_bir_lowering=False)

    hsT = nc.dram_tensor("hsT", [128, NK * S], BF16, kind="ExternalInput")
    wqT = nc.dram_tensor("wqT", [128, NK * GD], BF16, kind="ExternalInput")
    wkT = nc.dram_tensor("wkT", [128, NK * GD], BF16, kind="ExternalInput")
    wvT = nc.dram_tensor("wvT", [128, NK * GD], BF16, kind="ExternalInput")
    woT = nc.dram_tensor("woT", [128, 4 * E], BF16, kind="ExternalInput")
    cos4d = nc.dram_tensor("cos4", [128, NB, HL, ROT], BF16, kind="ExternalInput")
    sin4d = nc.dram_tensor("sin4", [128, NB, HL, ROT], BF16, kind="ExternalInput")
    masksd = nc.dram_tensor("masks", [128, 4, 512], BF16, kind="ExternalInput")
    vscaled = nc.dram_tensor("vscale", [128, NB, HL], F32, kind="ExternalInput")
    qmaskd = nc.dram_tensor("qmask", [128, NB], F32, kind="ExternalInput")
    identd = nc.dram_tensor("ident", [128, 128], BF16, kind="ExternalInput")
    outd = nc.dram_tensor("out", [S, E], BF16, kind="ExternalOutput")

    with TileContext(nc) as tc:
        from contextlib import ExitStack
        ctx = ExitStack()
        with ctx:
            const = ctx.enter_context(tc.tile_pool(name="const", bufs=1))
            qkT_pool = ctx.enter_context(tc.tile_pool(name="qkT", bufs=1))
            vn_pool = ctx.enter_context(tc.tile_pool(name="vn", bufs=1))
            scr = ctx.enter_context(tc.tile_pool(name="scr", bufs=4))
            rot_pool = ctx.enter_context(tc.tile_pool(name="rot", bufs=5))
            # two 4-bank PSUM rings shared by all phases
            psA = ctx.enter_context(tc.tile_pool(name="psA", bufs=4, space="PSUM"))
            psB = ctx.enter_context(tc.tile_pool(name="psB", bufs=4, space="PSUM"))

            cos4 = const.tile([128, NB, HL, ROT], BF16)
            sin4 = const.tile([128, NB, HL, ROT], BF16)
            masks = const.tile([128, 4, 512], BF16)
            vscale = const.tile([128, NB, HL], F32)
            qmask = const.tile([128, NB], F32)
            ident = const.tile([128, 128], BF16)
            # consts on the gpsimd DMA queue; weights go on the scalar
            # queue and hs on the sync queue so the three streams enqueue
            # in parallel.
            nc.gpsimd.dma_start(out=ident[:], in_=identd[:])
            nc.gpsimd.dma_start(out=qmask[:], in_=qmaskd[:])
            nc.gpsimd.dma_start(out=vscale[:], in_=vscaled[:])
            nc.gpsimd.dma_start(out=cos4[:], in_=cos4d[:])
            nc.gpsimd.dma_start(out=sin4[:], in_=sin4d[:])
            nc.gpsimd.dma_start(out=masks[:], in_=masksd[:])

            # HAM warmup: keep PE busy on dummy matmuls over an on-chip
            # constant (no DMA dependency) so the clock gate opens to
            # 2.4 GHz before the DMA-paced Q-projection starts.
            ones = nc.const_aps.scalar_like(1.0, qmask[:, 0:1])
            warm_ps = psB.tile([128, 128], F32, tag="ps")
            for _ in range(48):
                nc.tensor.matmul(warm_ps[0:1, 0:1], ones, ones,
                                 start=True, stop=True)

            # persistent transposed q/k: per local head, [hd=128, S]
            qT = [qkT_pool.tile([128, S], BF16, name=f"qT{h}") for h in range(HL)]
            kT = [qkT_pool.tile([128, S], BF16, name=f"kT{h}") for h in range(HL)]
            # v in natural layout per s-block: [128, 512]
            vn = [vn_pool.tile([128, GD], BF16, name=f"vn{m}") for m in range(NB)]
            # attention output (transposed) per head: [hd=128, S]
            aT = [qkT_pool.tile([128, S], BF16, name=f"aT{h}") for h in range(HL)]

            with tc.tile_pool(name="hs", bufs=1) as hs_pool, \
                 tc.tile_pool(name="w", bufs=1) as w_pool, \
                 tc.tile_pool(name="wo", bufs=4) as wo_pool, \
                 tc.tile_pool(name="atn", bufs=12) as atn_pool, \
                 tc.tile_pool(name="ost", bufs=2) as ost_pool:
                import concourse.bass as bass
                hs = hs_pool.tile([128, NK * S], BF16)

                # hs + wq interleaved in need-order on the sync queue, as
                # 4-k-slice chunks; dram is pre-packed in SBUF layout so
                # every DMA is 2D-contiguous (cheap descriptor generation)
                wqt = w_pool.tile([128, NK, GD], BF16, name="wqt", tag="w")
                for j in range(8):
                    nc.sync.dma_start(out=hs[:, j * 2 * S:(j + 1) * 2 * S],
                                      in_=hsT[:, j * 2 * S:(j + 1) * 2 * S])
                    nc.sync.dma_start(out=wqt[:, 2 * j:2 * (j + 1)],
                                      in_=wqT[:, j * 2 * GD:(j + 1) * 2 * GD])
                # then K/V/O weights, in need-order on the same queue
                wkt = w_pool.tile([128, NK, GD], BF16, name="wkt", tag="w")
                nc.sync.dma_start(out=wkt[:], in_=wkT[:])
                wvt = w_pool.tile([128, NK, GD], BF16, name="wvt", tag="w")
                nc.sync.dma_start(out=wvt[:], in_=wvT[:])
                wot = wo_pool.tile([128, 4, 4, 512], BF16, name="wot")
                nc.sync.dma_start(out=wot[:], in_=woT[:])
                wo_tiles = [[wot[:, n, kk] for kk in range(4)] for n in range(4)]
                wq = [wqt[:, k] for k in range(NK)]
                wk = [wkt[:, k] for k in range(NK)]
                wv = [wvt[:, k] for k in range(NK)]

                def proj_mms(wtiles, m, pool):
                    ps = pool.tile([128, GD], F32, name="ps_proj", tag="ps")
                    for k in range(NK):
                        nc.tensor.matmul(
                            ps[:], hs[:, k * S + m * 128: k * S + (m + 1) * 128],
                            wtiles[k], start=(k == 0), stop=(k == NK - 1))
                    return ps

                def qk_postproc(ps, m):
                    # sum-of-squares per head (rotary is norm-preserving, so
                    # norms come pre-rotary, straight from PSUM)
                    ss = scr.tile([128, HL], F32, tag="ss")
                    sqs = scr.tile([128, 128], F32, tag="sqs", bufs=1)
                    for h in range(HL):
                        nc.scalar.activation(out=sqs[:],
                                             in_=ps[:, h * 128:(h + 1) * 128],
                                             func=SQUARE, accum_out=ss[:, h:h + 1])
                    nrm = scr.tile([128, HL], F32, tag="nrm")
                    nc.scalar.sqrt(nrm[:], ss[:])
                    rr = scr.tile([128, HL], F32, tag="rr")
                    nc.vector.reciprocal(rr[:], nrm[:])
                    nc.vector.tensor_scalar_mul(rr[:], rr[:], qmask[:, m:m + 1])
                    # evict PSUM -> SBUF (bf16) with the per-row scale
                    # folded in, on Vector (Scalar is the postproc pacer)
                    qn = rot_pool.tile([128, HL, 128], BF16, tag="qn", bufs=16)
                    for h in range(HL):
                        nc.vector.tensor_scalar_mul(
                            qn[:, h], ps[:, h * 128:(h + 1) * 128], rr[:, h:h + 1])
                    # GPT-J interleaved rotary on first ROT dims of each head
                    qrot = rot_pool.tile([128, HL, ROT], BF16, tag="qrot", bufs=2)
                    tmp2 = rot_pool.tile([128, HL, ROT], BF16, tag="tmp2", bufs=2)
                    nc.gpsimd.tensor_tensor(out=qrot[:, :, 0:ROT:2], in0=qn[:, :, 1:ROT:2],
                                            in1=sin4[:, m, :, 0:ROT:2], op=MUL)
                    nc.vector.tensor_tensor(out=qrot[:, :, 1:ROT:2], in0=qn[:, :, 0:ROT:2],
                                            in1=sin4[:, m, :, 1:ROT:2], op=MUL)
                    nc.gpsimd.tensor_tensor(out=tmp2[:], in0=qn[:, :, 0:ROT],
                                            in1=cos4[:, m], op=MUL)
                    nc.gpsimd.tensor_add(out=qn[:, :, 0:ROT], in0=qrot[:], in1=tmp2[:])
                    return (qn, None)

                def transpose_block(qndg, m, dstT, pool):
                    qn, dg = qndg
                    for h in range(HL):
                        pt = pool.tile([128, 128], BF16, name="pt", tag="ps")
                        nc.tensor.transpose(pt[:], qn[:, h], ident[:])
                        nc.vector.tensor_copy(dstT[h][:, m * 128:(m + 1) * 128], pt[:])

                # ---- Q projection: two k-outer sweeps of 4 m-blocks each,
                # so the PE consumes hs/wq tiles as the DMA stream lands.
                qns = {}
                ps1 = [psA.tile([128, GD], F32, name=f"ps1_{m}", tag="ps")
                       for m in range(4)]
                for k in range(NK):
                    for m in range(4):
                        nc.tensor.matmul(
                            ps1[m][:], hs[:, k * S + m * 128: k * S + (m + 1) * 128],
                            wq[k], start=(k == 0), stop=(k == NK - 1))
                for m in range(4):
                    qns[m] = qk_postproc(ps1[m], m)
                for m in range(4, NB):
                    qns[m] = qk_postproc(proj_mms(wq, m, psB), m)
                # ---- K projection (transposes for both Q and K deferred
                # until after all K matmuls: the PE fills the postproc
                # latency window with matmuls instead of idling)
                kns = {}
                for m in range(NB):
                    kns[m] = qk_postproc(proj_mms(wk, m, psA), m)
                for m in range(NB):
                    transpose_block(qns.pop(m), m, qT, psB)
                for m in range(NB - 3):
                    transpose_block(kns.pop(m), m, kT, psB)
                for m in range(NB - 3, NB):
                    transpose_block(kns.pop(m), m, kT, psB)

                # ---- interleaved V-projection / attention / out-projection,
                # one 512-query half at a time
                for c in range(2):
                    # V-proj for s-blocks 4c..4c+3
                    for m in range(4 * c, 4 * c + 4):
                        ps = proj_mms(wv, m, psB)
                        for h in range(HL):
                            if h < 2:
                                nc.vector.tensor_scalar_mul(
                                    vn[m][:, h * 128:(h + 1) * 128],
                                    ps[:, h * 128:(h + 1) * 128],
                                    vscale[:, m, h:h + 1])
                            else:
                                nc.scalar.mul(
                                    vn[m][:, h * 128:(h + 1) * 128],
                                    ps[:, h * 128:(h + 1) * 128],
                                    vscale[:, m, h:h + 1])
                    # attention for query half c, all local heads
                    nj = 4 * (c + 1)
                    for h in range(HL):
                        at_tiles = []
                        for j in range(nj):
                            pa = psA.tile([128, 512], F32, name="pa", tag="ps")
                            nc.tensor.matmul(pa[:], kT[h][:, j * 128:(j + 1) * 128],
                                             qT[h][:, c * 512:(c + 1) * 512],
                                             start=True, stop=True)
                            at = atn_pool.tile([128, 512], BF16, tag="at", name="at")
                            d = j - 4 * c
                            if d >= 0:   # diagonal band: apply causal mask
                                nc.vector.tensor_tensor(out=at[:], in0=pa[:],
                                                        in1=masks[:, d], op=MUL)
                            else:         # below diagonal: plain evict
                                nc.scalar.copy(at[:], pa[:])
                            at_tiles.append(at)
                        po = psB.tile([128, 512], F32, name="po", tag="ps")
                        for j in range(nj):
                            nc.tensor.matmul(po[:], vn[j][:, h * 128:(h + 1) * 128],
                                             at_tiles[j][:],
                                             start=(j == 0), stop=(j == nj - 1))
                        nc.scalar.copy(aT[h][:, c * 512:(c + 1) * 512], po[:])
                    # out-projection for s-blocks 4c..4c+3
                    for m in range(4 * c, 4 * c + 4):
                        ot = ost_pool.tile([128, E], BF16, tag="ot", name="ot")
                        for n in range(4):
                            pool = psA if n % 2 == 0 else psB
                            ps = pool.tile([128, 512], F32, name="ps_o", tag="ps")
                            for k in range(HL):
                                nc.tensor.matmul(ps[:], aT[k][:, m * 128:(m + 1) * 128],
                                                 wo_tiles[n][k],
                                                 start=(k == 0), stop=(k == HL - 1))
                            if n % 2 == 0:
                                nc.vector.tensor_copy(ot[:, n * 512:(n + 1) * 512], ps[:])
                            else:
                                nc.scalar.copy(ot[:, n * 512:(n + 1) * 512], ps[:])
                            # stream each quarter out as soon as it evicts
                            q = nc.sync if n % 2 == 0 else nc.gpsimd
                            q.dma_start(
                                out=outd[m * 128:(m + 1) * 128,
                                         n * 512:(n + 1) * 512],
                                in_=ot[:, n * 512:(n + 1) * 512])

    nc.compile()
    _BUILT = nc
    return nc


def _prep_inputs(hidden_states, w_q, w_k, w_v, w_o, norm_const,
                 attention_mask, position_ids):
    """Host-side shard + table prep. Returns list of 8 in_maps."""
    hidden_states = np.asarray(hidden_states, dtype=np.float32)
    w_q = np.asarray(w_q, dtype=np.float32)
    w_k = np.asarray(w_k, dtype=np.float32)
    w_v = np.asarray(w_v, dtype=np.float32)
    w_o = np.asarray(w_o, dtype=np.float32)
    norm_const = np.asarray(norm_const, dtype=np.float32).reshape(H)
    attention_mask = np.asarray(attention_mask, dtype=np.float32).reshape(B, S)
    position_ids = np.asarray(position_ids).reshape(B, S).astype(np.int64)

    embed = _sinusoidal(MAXP, ROT)                       # [MAXP, 64]
    sig = 1.0 / (1.0 + np.exp(-norm_const.astype(np.float64)))   # [H]
    mask0 = (attention_mask == 0).astype(np.float32)     # [B, S]
    counts = np.cumsum(mask0, axis=1).astype(np.float32)  # [B, S]
    denom = np.maximum(counts[:, None, :] ** sig[None, :, None], 1.0).astype(np.float32)
    vs_full = mask0[:, None, :] / denom                  # [B, H, S]

    # causal masks for the 4 diagonal-band block offsets (512-query cols)
    p = np.arange(128)[:, None]
    f = np.arange(512)[None, :]
    masks = np.stack([(d * 128 + p <= f) for d in range(4)]).astype(BF16NP)
    masks = np.ascontiguousarray(masks.transpose(1, 0, 2))  # [128, 4, 512]
    ident = np.eye(128, dtype=BF16NP)

    in_maps = []
    for b in range(B):
        sincos = embed[position_ids[b]]                  # [S, 64]
        sin, cos = sincos[:, :ROT // 2], sincos[:, ROT // 2:]
        cosR = np.repeat(cos, 2, axis=1)                 # [S, 64]
        sinS = np.empty((S, ROT), dtype=np.float32)
        sinS[:, 0::2] = -sin
        sinS[:, 1::2] = sin
        # [S,64] -> [128 part, NB, 64] -> broadcast over HL heads
        def to4(t):
            t = t.reshape(NB, 128, ROT).transpose(1, 0, 2)
            return np.ascontiguousarray(
                np.broadcast_to(t[:, :, None, :], (128, NB, HL, ROT))).astype(BF16NP)
        cos4 = to4(cosR)
        sin4 = to4(sinS)
        qm = np.ascontiguousarray(mask0[b].reshape(NB, 128).T)  # [128, NB]
        # pack [E, S] -> [128, NK*S] (partition-major SBUF layout)
        hsT_b = np.ascontiguousarray(
            hidden_states[b].T.reshape(NK, 128, S).transpose(1, 0, 2)
            .reshape(128, NK * S)).astype(BF16NP)

        def packw(wT):  # [E, GD] -> [128, NK*GD]
            return np.ascontiguousarray(
                wT.reshape(NK, 128, GD).transpose(1, 0, 2)
                .reshape(128, NK * GD)).astype(BF16NP)

        for g in range(4):
            sl = slice(g * GD, (g + 1) * GD)
            vs = vs_full[b, 4 * g:4 * g + HL, :]                # [HL, S]
            vs = np.ascontiguousarray(
                vs.reshape(HL, NB, 128).transpose(2, 1, 0))     # [128, NB, HL]
            # wo: [GD, E] -> [128, n, kk, 512] -> [128, 4*E]
            woT_g = w_o[:, sl].T                                # [GD, E]
            wo_p = np.ascontiguousarray(
                woT_g.reshape(4, 128, 4, 512).transpose(1, 2, 0, 3)
                .reshape(128, 4 * E)).astype(BF16NP)
            in_maps.append({
                "hsT": hsT_b,
                "wqT": packw(w_q[sl, :].T),
                "wkT": packw(w_k[sl, :].T),
                "wvT": packw(w_v[sl, :].T),
                "woT": wo_p,
                "cos4": cos4, "sin4": sin4, "masks": masks,
                "vscale": vs, "qmask": qm, "ident": ident,
            })
    # core order: c = b*4 + g
    return in_maps


def run(inputs, trace=False, trace_cores=None):
    from concourse.bass_utils import run_bass_kernel_spmd
    nc = _build()
    in_maps = _prep_inputs(**inputs)
    res = run_bass_kernel_spmd(nc, in_maps, core_ids=list(range(8)),
                               trace=trace, trace_cores=trace_cores)
    partials = [np.asarray(res.results[c]["out"], dtype=np.float32)
                for c in range(8)]
    out = np.empty((B, S, E), dtype=np.float32)
    for b in range(B):
        out[b] = partials[4 * b] + partials[4 * b + 1] \
            + partials[4 * b + 2] + partials[4 * b + 3]
    return out, res


def kernel(**inputs):
    out, _ = run(inputs, trace=False)
    return out
